# revision 48
# baseline (speedup 1.0000x reference)
"""Trainium2 Bass kernel for nn_DepthRenderer (superquadric depth renderer).

Sharding: rows round-robin over 8 cores (core c owns image rows r = 8*lr+c,
lr=0..44).  Per-core layout [128 lanes, 45 lrows, 5 xblocks]; lane = x%128,
xblock = x//128.  Each core renders all 8 SQs (constants baked as immediates
into one SPMD program) and min-accumulates depth on device; host concatenates.

Sparsity: a SQ can only influence pixels where the ray enters its bounding
sphere: h(d) = (b.d)^2 - (C-3) * d^T A d > 0 (homogeneous quadratic in the ray
direction, so normalization-free).  The host evaluates h on a coarse pixel
subgrid, takes the bounding rectangle (+margin, rows rounded to multiples of 8
so the rect is the SAME static view on every core), and the device program
processes only that rect per SQ (~4.7x less work).  Pixels outside the rect
keep depth FAR; rect pixels use the exact in-rect mask, and the premask
boundary is depth-continuous (grazing rays integrate to ~FAR), so the coarse
rect is safe.

Math notes (exact rewrites of the reference, up to fp rounding):
  - a == sizes  =>  X = |loc|/a + eps = |pts_loc| + eps  (sizes cancel)
  - ||td * sizes|| = ||d|| * rinv  (rotation invariance)
  - dt0  = ||pts_loc[0]*s + R^T p||,  dt10 = ||(PL10-PL9)*s||  with
    PL10 = loc_far/s = 1.5*u - (R^T p)/s
  - sqrt(x) = exp(0.5*ln(x)); sigmoid(x) = 0.5 + 0.5*tanh(x/2)
  - phase 1 (pow chains) uses the natural_log_exp ACT table set, phase 2
    (tanh occupancy + visibility exp) uses exp_and_others; both loads are
    pre-placed so bacc inserts no further table switches.
"""

from contextlib import ExitStack

import numpy as np

import concourse.bass as bass
import concourse.bacc as bacc
import concourse.mybir as mybir
from concourse import tile
from concourse.bass_utils import run_bass_kernel_spmd

F32 = mybir.dt.float32
AF = mybir.ActivationFunctionType
OP = mybir.AluOpType

# renderer constants (match the nn.Module init)
HS, WS = 360, 640
NEAR, FAR = 0.0, 1.5
NS = 10
SHARP = 1000.0
TAU = 100.0
N_SQ = 8
EPS = 1e-6

N_CORES = 8
NRL = HS // N_CORES       # 45 local rows per core
NJ = WS // 128            # 5 x-blocks
NCOL = NRL * NJ           # 225 columns per core
P = 128


def _f(x):
    return float(np.float32(x))


def _host_consts(sq_poses, sq_params, rays_o, t):
    """Per-SQ scalars, computed in float64 from the f32 inputs."""
    sq_poses = np.asarray(sq_poses, np.float64)
    sq_params = np.asarray(sq_params, np.float64)
    rays_o = np.asarray(rays_o, np.float64)
    t = np.asarray(t, np.float64)

    consts = []
    for k in range(N_SQ):
        R = sq_poses[k, :3, :3]
        p = sq_poses[k, :3, 3]
        s = sq_params[k, 0:3]
        e1 = sq_params[k, 3]
        e2 = sq_params[k, 4]

        M1 = R.T / s[:, None]            # u = M1 @ d = (R^T d)/s
        tc = (R.T @ (rays_o - p)) / s
        rp = R.T @ p                      # loc(near) = -rp
        rps = rp / s
        c1 = 2.0 / e2
        c2 = e2 / e1
        c3 = 2.0 / e1

        # near-point occupancy (constant per SQ)
        Xn = np.abs(-rp) / s + EPS
        fN = (Xn[0] ** c1 + Xn[1] ** c1) ** c2 + Xn[2] ** c3
        Fn = fN ** e1
        with np.errstate(over="ignore"):
            occ0 = 1.0 / (1.0 + np.exp(-SHARP * (1.0 - Fn)))
        vis0 = np.exp(-TAU * occ0)

        consts.append(dict(
            M1=M1, tc=tc, rp=rp, rps=rps, s=s,
            c1=c1, c2=c2, c3=c3, e1=e1,
            occ0=occ0, vis0=vis0,
        ))

    # segment weights from t (shared across SQs)
    dt_abs = np.abs(np.diff(t))          # |t_i - t_{i-1}|, i=1..9
    beta = np.zeros(11)                  # weight of v_s (s=1..10) in inner sum
    for i in range(1, NS):               # inner gaps i=1..9 use v_i, v_{i+1}
        beta[i] += 0.5 * dt_abs[i - 1]
        beta[i + 1] += 0.5 * dt_abs[i - 1]
    return consts, t, beta


def _host_rects(consts, rays_d):
    """Per-SQ (lr0, nr, j0, nj) bounding rect, identical across cores.

    h(d) = (b.d)^2 - (C-3) d^T A d is degree-2 homogeneous in d, so the
    coarse-subgrid sign test needs no ray normalization.  Conservative by a
    9px margin (>> 3px grid step; min blob diameter is ~40px for any SQ with
    C comfortably > 3).  Rows rounded to multiples of 8 so that every core's
    local-row range is the same [lr0, lr0+nr).
    """
    d = np.asarray(rays_d, np.float64)
    ys = np.arange(0, HS, 2)
    xs = np.arange(0, WS, 2)
    sub = d[np.ix_(ys, xs)]
    rects = []
    for cc in consts:
        M1, tcv = cc["M1"], cc["tc"]
        C = float((tcv ** 2).sum())
        if C <= 3.5:                      # near/inside bounding sphere: dense
            rects.append((0, NRL, 0, NJ))
            continue
        A = M1.T @ M1
        b = M1.T @ tcv
        hq = (sub @ b) ** 2 - (C - 3.0) * np.einsum("yxi,ij,yxj->yx", sub, A, sub)
        hit = hq > 0
        if not hit.any():
            rects.append(None)
            continue
        ryy, rxx = np.where(hit)
        r0 = max(0, int(ys[ryy.min()]) - 3)
        r1 = min(HS - 1, int(ys[ryy.max()]) + 3)
        x0 = max(0, int(xs[rxx.min()]) - 3)
        x1 = min(WS - 1, int(xs[rxx.max()]) + 3)
        r0 = (r0 // 8) * 8
        r1 = min(HS, ((r1 + 8) // 8) * 8) - 1
        lr0, nr = r0 // 8, (r1 - r0 + 1) // 8
        j0, j1 = x0 // 128, x1 // 128
        rects.append((lr0, nr, j0, j1 - j0 + 1))
    return rects


def build_program(consts, t, beta, rects, act_loads=True):
    """One SPMD program; input rdin [128,3,45,5], output depth [128,45,5]."""
    nc = bacc.Bacc("TRN2", target_bir_lowering=False, debug=False,
                   enable_asserts=False, num_devices=N_CORES)

    rd_dram = nc.dram_tensor("rdin", [P, 3, NRL, NJ], F32, kind="ExternalInput")
    out_dram = nc.dram_tensor("depth", [P, NRL, NJ], F32, kind="ExternalOutput")

    # const APs for activation biases (only 0.0/1.0 are pre-registered)
    def reg_const(v):
        v = _f(v)
        if (F32, v) not in nc.const_aps.aps:
            th = nc.alloc_sbuf_tensor(f"constap{len(nc.const_aps.aps)}", [128, 1], F32)
            nc.gpsimd.memset(th.ap(), v)
            nc.const_aps.aps[(F32, v)] = th.ap()

    reg_const(EPS)
    reg_const(-SHARP)
    for cc in consts:
        for j in range(3):
            reg_const(cc["rp"][j])
            reg_const(cc["tc"][j])
    nc.all_engine_barrier()

    live = [k for k in range(N_SQ) if rects[k] is not None]

    with tile.TileContext(nc) as tc, ExitStack() as es:
        V = nc.vector
        S = nc.scalar
        persist = es.enter_context(tc.tile_pool(name="persist", bufs=1))

        # ---- shared loads & per-core shared prep ----
        rd = persist.tile([P, 3, NRL, NJ], F32, name="rd")
        nc.sync.dma_start(rd[:, :, :, :], rd_dram.ap())

        rdsq = persist.tile([P, 3, NRL, NJ], F32, name="rdsq")
        S.activation(rdsq[:, :, :, :], rd[:, :, :, :], AF.Square)
        nd2 = persist.tile([P, NRL, NJ], F32, name="nd2")
        V.tensor_tensor(nd2[:, :, :], rdsq[:, 0, :, :], rdsq[:, 1, :, :], OP.add)
        V.tensor_tensor(nd2[:, :, :], nd2[:, :, :], rdsq[:, 2, :, :], OP.add)
        nd = persist.tile([P, NRL, NJ], F32, name="nd")
        S.activation(nd[:, :, :], nd2[:, :, :], AF.Ln)
        S.activation(nd[:, :, :], nd[:, :, :], AF.Exp, scale=0.5)

        dmin = persist.tile([P, NRL, NJ], F32, name="dmin")
        V.memset(dmin[:, :, :], FAR)

        XMAX = max((r[1] * r[3] for r in rects if r is not None), default=1)
        betaE = persist.tile([P, 5, XMAX], F32, name="betaE")
        betaO = persist.tile([P, 5, XMAX], F32, name="betaO")
        for i in range(5):
            V.memset(betaE[:, i, :], _f(beta[2 * i + 1]))   # s = 0,2,4,6,8
            V.memset(betaO[:, i, :], _f(beta[2 * i + 2]))   # s = 1,3,5,7,9

        # persistent per-SQ results for phase 2 (sized per rect)
        FF, HG, DTT = {}, {}, {}
        for k in live:
            lr0, nr, j0, nj = rects[k]
            X = nr * nj
            FF[k] = persist.tile([P, NS + 1, X], F32, name=f"FF{k}")
            HG[k] = persist.tile([P, X], F32, name=f"HG{k}")
            DTT[k] = persist.tile([P, 2, X], F32, name=f"DTT_{k}")

        # ---------------- phase 1: per-SQ F chains (ln/exp table set) -------
        with tc.tile_pool(name="p1", bufs=3) as pool:
            for k in live:
                cc = consts[k]
                E = V
                lr0, nr, j0, nj = rects[k]
                X = nr * nj
                M1, tcv, rp, rps, s = cc["M1"], cc["tc"], cc["rp"], cc["rps"], cc["s"]

                def r4(ap2):   # [P, X] compact view -> [P, nr, nj]
                    return ap2.rearrange("p (a b) -> p a b", b=nj)

                # read the rect views of rd directly (outs reshaped to match)
                rv = [rd[:, jj, lr0:lr0 + nr, j0:j0 + nj] for jj in range(3)]
                u = pool.tile([P, 3, X], F32, tag="u")
                for j in range(3):
                    uo = r4(u[:, j, :])
                    E.tensor_scalar(uo, rv[0], _f(M1[j, 0]), None, OP.mult)
                    E.scalar_tensor_tensor(uo, rv[1], _f(M1[j, 1]), uo, OP.mult, OP.add)
                    E.scalar_tensor_tensor(uo, rv[2], _f(M1[j, 2]), uo, OP.mult, OP.add)

                usq = pool.tile([P, 3, X], F32, tag="usq")
                E.tensor_tensor(usq[:, :, :], u[:, :, :], u[:, :, :], OP.mult)
                nu2 = pool.tile([P, X], F32, tag="nu2")
                E.tensor_tensor(nu2[:], usq[:, 0, :], usq[:, 1, :], OP.add)
                E.tensor_tensor(nu2[:], nu2[:], usq[:, 2, :], OP.add)

                # 1/nu2 on the vector engine (2-ULP approx) replaces the
                # rinv=exp(-0.5 ln nu2) ACT round-trip: cen = tc + (|tc.u|/nu2)u
                rq = pool.tile([P, X], F32, tag="rq")
                rqs = pool.tile([P, X], F32, tag="rqs")
                E.reciprocal_approx_accurate(rq[:], nu2[:], rqs[:])

                # d1 = -tc.u (>=0 for any real hit); q = max(d1,0)/nu2 in one
                # fused STT -- mirror-cone pixels degenerate to cen=tc (-> FAR)
                d1 = pool.tile([P, X], F32, tag="d1")
                E.tensor_scalar(d1[:], u[:, 0, :], _f(-tcv[0]), None, OP.mult)
                E.scalar_tensor_tensor(d1[:], u[:, 1, :], _f(-tcv[1]), d1[:], OP.mult, OP.add)
                E.scalar_tensor_tensor(d1[:], u[:, 2, :], _f(-tcv[2]), d1[:], OP.mult, OP.add)
                proj = pool.tile([P, X], F32, tag="proj")
                E.scalar_tensor_tensor(proj[:], d1[:], 0.0, rq[:], OP.max, OP.mult)

                cen = pool.tile([P, 3, X], F32, tag="cen")
                E.tensor_tensor(cen[:, :, :], proj[:].unsqueeze(1).broadcast_to((P, 3, X)),
                                u[:, :, :], OP.mult)
                for j in range(3):
                    E.tensor_scalar(cen[:, j, :], cen[:, j, :], _f(tcv[j]), None, OP.add)

                csq = pool.tile([P, 3, X], F32, tag="usq")
                E.tensor_tensor(csq[:, :, :], cen[:, :, :], cen[:, :, :], OP.mult)
                m3 = pool.tile([P, X], F32, tag="m3")
                E.tensor_tensor(m3[:], csq[:, 0, :], csq[:, 1, :], OP.add)
                E.tensor_tensor(m3[:], m3[:], csq[:, 2, :], OP.add)
                # m3 = 3 - dist^2 ; mask = m3 > 0 ; hclsq = max(m3, 1e-12)
                E.tensor_scalar(m3[:], m3[:], -1.0, 3.0, OP.mult, OP.add)
                E.tensor_scalar(m3[:], m3[:], 1e-12, None, OP.max)

                # w = hcl*rinv = sqrt((3-dist^2)/nu2); htd = w*u; hg = ||d||*w
                hcl = pool.tile([P, X], F32, tag="hcl")
                E.tensor_tensor(hcl[:], m3[:], rq[:], OP.mult)
                S.activation(hcl[:], hcl[:], AF.Ln)
                S.activation(hcl[:], hcl[:], AF.Exp, scale=0.5)

                E.tensor_tensor(r4(HG[k][:]), nd[:, lr0:lr0 + nr, j0:j0 + nj],
                                r4(hcl[:]), OP.mult)

                htd = pool.tile([P, 3, X], F32, tag="htd")
                E.tensor_tensor(htd[:, :, :], hcl[:].unsqueeze(1).broadcast_to((P, 3, X)),
                                u[:, :, :], OP.mult)

                # PL slots 0..9: cen + t_s*htd ; slot 10: 1.5*u - rp/s
                PL = pool.tile([P, NS + 1, 3, X], F32, tag="PL", bufs=3)
                for si in range(NS):
                    E.scalar_tensor_tensor(PL[:, si, :, :], htd[:, :, :], _f(t[si]),
                                           cen[:, :, :], OP.mult, OP.add)
                for j in range(3):
                    E.tensor_scalar(PL[:, NS, j, :], u[:, j, :], 1.5, _f(-rps[j]),
                                    OP.mult, OP.add)

                # dt0 = ||PL0*s + rp|| ; dt10 = ||(PL10-PL9)*s||
                # All samples lie on the ray: sample s sits at world ray
                # parameter tau_s = d1/nu2 + w*t_s, so the boundary segment
                # lengths need no norms: dt0 = |tau_1|*||d||, and
                # dt10 = |1.5 - tau_last|*||d||.
                dtt = DTT[k]
                base = pool.tile([P, X], F32, tag="q3")
                E.tensor_tensor(base[:], d1[:], rq[:], OP.mult)
                tau = pool.tile([P, 2, X], F32, tag="q3b")
                E.scalar_tensor_tensor(tau[:, 0, :], hcl[:], _f(t[0]), base[:],
                                       OP.mult, OP.add)
                E.scalar_tensor_tensor(tau[:, 1, :], hcl[:], _f(t[NS - 1]), base[:],
                                       OP.mult, OP.add)
                E.tensor_scalar(tau[:, 1, :], tau[:, 1, :], -1.0, 1.5, OP.mult, OP.add)
                tneg = pool.tile([P, 2, X], F32, tag="tneg")
                E.tensor_scalar(tneg[:, :, :], tau[:, :, :], -1.0, None, OP.mult)
                E.tensor_tensor(tau[:, :, :], tau[:, :, :], tneg[:, :, :], OP.max)
                ndv = nd[:, lr0:lr0 + nr, j0:j0 + nj]
                E.tensor_tensor(r4(dtt[:, 0, :]), r4(tau[:, 0, :]), ndv, OP.mult)
                E.tensor_tensor(r4(dtt[:, 1, :]), r4(tau[:, 1, :]), ndv, OP.mult)

                # F chain, in place over PL
                flat = PL[:, :, :, :]
                S.activation(flat, flat, AF.Abs)                       # |PL|
                S.activation(flat, flat, AF.Ln, bias=_f(EPS))          # ln(|PL|+eps)
                S.activation(PL[:, :, 0:2, :], PL[:, :, 0:2, :], AF.Exp,
                             scale=_f(cc["c1"]))                       # u,v
                E.tensor_tensor(PL[:, :, 0, :], PL[:, :, 0, :], PL[:, :, 1, :], OP.add)
                S.activation(PL[:, :, 0, :], PL[:, :, 0, :], AF.Ln)
                S.activation(PL[:, :, 0, :], PL[:, :, 0, :], AF.Exp, scale=_f(cc["c2"]))
                S.activation(PL[:, :, 2, :], PL[:, :, 2, :], AF.Exp, scale=_f(cc["c3"]))
                E.tensor_tensor(PL[:, :, 0, :], PL[:, :, 0, :], PL[:, :, 2, :], OP.add)
                # clamp wz <= 1.088^(1/e1)  (== clamping F at 1.088, monotone)
                E.tensor_scalar(PL[:, :, 0, :], PL[:, :, 0, :],
                                _f(1.088 ** (1.0 / cc["e1"])), None, OP.min)
                S.activation(PL[:, :, 0, :], PL[:, :, 0, :], AF.Ln)
                S.activation(FF[k][:, :, :], PL[:, :, 0, :], AF.Exp, scale=_f(cc["e1"]))

                # ---- occupancy/visibility/depth (same ln/exp table set) ----
                # occ = sigmoid(1000*(1-F)) = 1/(1 + e^(1000F-1000)); F clamped
                # at 1.088 so e^x <= 1.65e38 (sigma there is 6e-39 ~ 0).
                occ = pool.tile([P, NS + 1, X], F32, tag="occ", bufs=3)
                S.activation(occ[:, :, :], FF[k][:, :, :], AF.Exp,
                             scale=SHARP, bias=-SHARP)
                S.activation(occ[:, :, :], occ[:, :, :], AF.Identity, bias=1.0)
                rscr = pool.tile([P, NS + 1, X], F32, tag="rscr")
                E.reciprocal_approx_fast(rscr[:, :, :], occ[:, :, :])

                # paired-prefix cumsum: po_i = oc_2i+oc_2i+1 -> prefix over
                # pairs gives odd cums; evens = po_shift + oc_even (1 op)
                E.tensor_scalar(rscr[:, 0, :], rscr[:, 0, :], _f(cc["occ0"]), None, OP.add)
                po = pool.tile([P, 5, X], F32, tag="cum", bufs=3)
                E.tensor_tensor(po[:, :, :], rscr[:, 0:NS:2, :], rscr[:, 1:NS + 1:2, :], OP.add)
                for i in range(1, 5):
                    E.tensor_tensor(po[:, i, :], po[:, i - 1, :], po[:, i, :], OP.add)
                cue = pool.tile([P, 6, X], F32, tag="cue", bufs=3)
                E.tensor_copy(cue[:, 0, :], rscr[:, 0, :])
                E.tensor_tensor(cue[:, 1:6, :], po[:, :, :], rscr[:, 2:NS + 1:2, :], OP.add)
                S.activation(po[:, :, :], po[:, :, :], AF.Exp, scale=-TAU)   # v odd
                S.activation(cue[:, :, :], cue[:, :, :], AF.Exp, scale=-TAU)  # v even

                acc = pool.tile([P, X], F32, tag="acc")
                wv = pool.tile([P, NS, X], F32, tag="wv")
                E.tensor_tensor(wv[:, 0:5, :], cue[:, 0:5, :], betaE[:, :, 0:X], OP.mult)
                E.tensor_tensor(wv[:, 5:10, :], po[:, :, :], betaO[:, :, 0:X], OP.mult)
                s1 = pool.tile([P, 5, X], F32, tag="s1")
                E.tensor_tensor(s1[:, :, :], wv[:, 0:5, :], wv[:, 5:10, :], OP.add)
                E.tensor_tensor(s1[:, 0:2, :], s1[:, 0:2, :], s1[:, 2:4, :], OP.add)
                E.tensor_tensor(acc[:], s1[:, 0, :], s1[:, 1, :], OP.add)
                E.tensor_tensor(acc[:], acc[:], s1[:, 4, :], OP.add)
                E.tensor_tensor(acc[:], acc[:], HG[k][:], OP.mult)

                b1 = pool.tile([P, X], F32, tag="b1")
                E.tensor_scalar(b1[:], cue[:, 0, :], 0.5, _f(0.5 * cc["vis0"]),
                                OP.mult, OP.add)
                E.tensor_tensor(b1[:], b1[:], DTT[k][:, 0, :], OP.mult)
                E.tensor_tensor(acc[:], acc[:], b1[:], OP.add)

                b2 = pool.tile([P, X], F32, tag="b2")
                E.tensor_tensor(b2[:], po[:, 4, :], cue[:, 5, :], OP.add)
                E.scalar_tensor_tensor(b2[:], b2[:], 0.5, DTT[k][:, 1, :], OP.mult, OP.mult)
                E.tensor_tensor(acc[:], acc[:], b2[:], OP.add)

                # masked-out rect pixels integrate to 1.5 +- 1e-6 == FAR
                # (F > 1 strictly outside the bounding sphere => vis == 1,
                # and the sample polyline is monotone on the ray), so the
                # explicit mask/select is unnecessary: min() absorbs them.
                dv = dmin[:, lr0:lr0 + nr, j0:j0 + nj]
                V.tensor_tensor(dv, dv, acc[:].rearrange("p (a b) -> p a b", b=nj),
                                OP.min)


        nc.sync.dma_start(out_dram.ap(), dmin[:, :, :])

    # Pre-place the two ACT table loads (natural_log_exp for phase 1,
    # exp_and_others for phase 2/tanh) so bacc's fixpoint inserts none.
    # (CoreSim can't handle the hand-inserted loads; act_loads=False skips.)
    if not act_loads:
        nc.compile()
        return nc
    from concourse.hw_specs import get_activation_tables
    names = list(get_activation_tables(nc.m.arch).keys())
    id_nle = names.index("natural_log_exp_and_others")

    def make_load(set_id):
        ins = mybir.InstLoadActFuncSet(
            name=nc.get_next_instruction_name(), act_func_set_id=set_id,
            ins=[], outs=[])
        ins.engine = nc.scalar.engine
        return ins

    for blk in nc.main_func.blocks:
        il = blk.instructions
        first_act = next((i for i, x in enumerate(il)
                          if isinstance(x, mybir.InstActivation)), None)
        if first_act is None:
            continue
        il.insert(first_act, make_load(id_nle))

    nc.compile()
    return nc


def _shard_rays(rays_d):
    """-> per-core arrays [128, 3, 45, 5]; core c owns rows 8*lr+c."""
    rd = np.asarray(rays_d, np.float32)
    out = []
    for c in range(N_CORES):
        sub = rd[c::N_CORES]                         # (45, 640, 3)
        arr = sub.reshape(NRL, NJ, 128, 3).transpose(2, 3, 0, 1)
        out.append(np.ascontiguousarray(arr))        # (128, 3, 45, 5)
    return out


def _unshard(outs):
    """outs: list of 8 arrays [128, 45, 5] -> (360, 640)."""
    full = np.empty((HS, WS), np.float32)
    for c in range(N_CORES):
        full[c::N_CORES] = outs[c].transpose(1, 2, 0).reshape(NRL, WS)
    return full


def kernel(sq_poses, sq_params, rays_d, rays_o, t, **run_kwargs):
    consts, tv, beta = _host_consts(sq_poses, sq_params, rays_o, t)
    rects = _host_rects(consts, rays_d)
    nc = build_program(consts, tv, beta, rects)
    planes = _shard_rays(rays_d)
    in_maps = [{"rdin": planes[c]} for c in range(N_CORES)]
    res = run_bass_kernel_spmd(nc, in_maps, core_ids=list(range(N_CORES)), **run_kwargs)
    outs = [res.results[c]["depth"] for c in range(N_CORES)]
    out = _unshard(outs).astype(np.float32)
    kernel.last_result = res
    return out


kernel.last_result = None
